# revision 3
# baseline (speedup 1.0000x reference)
"""Trainium2 Bass kernel for masked spatial attention softmax.

Computes S = softmax((F_a@Wq.T + bq) @ (F_s@Wk.T + bk).T / sqrt(d) + mask)
over 8 NeuronCores, data-parallel over batch.

Algebra: QK = Q_a @ K_s.T = (F_a @ Wc + bc) @ F_s.T where
Wc = Wq.T @ Wk / sqrt(d), bc = bq @ Wk / sqrt(d) are folded on the host
(weights-only math); the bk term is constant along the softmax axis and
drops out of the softmax.  K_s is never materialized.

v2: F_s.T and F_a.T come straight from HBM via XBAR DMA-transpose
(InstDmaTransposeAnt, ~293 B/ns) instead of PE transposes + DVE
evictions.  PE runs only QK + mask + one small projection; exp writes
bf16 directly; row sums ride DVE tensor_reduce instead of ACT accum;
all DMA issues ride the Sync queue so Scalar does nothing but exp.
"""

import math
from contextlib import ExitStack

import numpy as np
import ml_dtypes

import concourse.bass as bass
import concourse.tile as tile
from concourse import bacc, mybir

# Problem shapes (hardcoded per contract; spec: B=32, T=256, HW=4096, d=256)
B_FULL = 32
N_CORES = 8
BS = B_FULL // N_CORES  # batches per core
T = 256
HW = 4096
D = 256
CK = 1024  # QK chunk width (2 PSUM banks)
NCK = HW // CK
SCALE = 1.0 / math.sqrt(D)  # 1/16
MASK_NEG = -80.0  # exp(-80 + max_logit) << 1e-30; stays in ACT exp valid range

F32 = mybir.dt.float32
BF16 = mybir.dt.bfloat16


def _build_body(tc, ctx, F_a, F_s, mbig, Wc, bc, S):
    nc = tc.nc

    singles = ctx.enter_context(tc.tile_pool(name="singles", bufs=1))
    fst_pool = ctx.enter_context(tc.tile_pool(name="fst", bufs=2))
    qpool = ctx.enter_context(tc.tile_pool(name="qpool", bufs=2))
    spool = ctx.enter_context(tc.tile_pool(name="spool", bufs=2))
    opool = ctx.enter_context(tc.tile_pool(name="opool", bufs=2))
    stats = ctx.enter_context(tc.tile_pool(name="stats", bufs=4))
    psum_qk = ctx.enter_context(tc.tile_pool(name="psum_qk", bufs=3, space="PSUM"))
    psum_pj = ctx.enter_context(tc.tile_pool(name="psum_pj", bufs=2, space="PSUM"))

    # ---- small constants ----
    ones16 = singles.tile([1, 128], BF16, tag="ones16", name="ones16")
    nc.vector.memset(ones16[:], 1.0)

    mb_sb = singles.tile([1, BS * HW], BF16, tag="mb", name="mb")  # 0 / MASK_NEG
    bc_sb = singles.tile([128, 2], F32, tag="bc", name="bc")

    fat_t, qct_t, fst_t = {}, {}, {}

    # ---- prologue loads, ordered for fastest time-to-first-matmul ----
    fat0 = qpool.tile([128, 2, T], BF16, tag="fat", name="fat")
    for ci in range(2):
        nc.sync.dma_start(
            out=fat0[:, ci, :], in_=F_a[0, :, ci * 128:(ci + 1) * 128],
            transpose=True,
        )
    fat_t[0] = fat0

    wc_sb = singles.tile([128, 2, D], BF16, tag="wc", name="wc")
    nc.sync.dma_start(out=wc_sb[:], in_=Wc.rearrange("(kh kl) o -> kl kh o", kl=128))
    nc.sync.dma_start(out=bc_sb[:], in_=bc.rearrange("(a p) -> p a", p=128))
    nc.sync.dma_start(out=mb_sb[:], in_=mbig.rearrange("b s -> (b s)")[None, :])

    # F_s[0].T: first chunk (both ci) ASAP, then the rest.
    fst0 = fst_pool.tile([128, 2, HW], BF16, tag="fst", name="fst")
    for ci in range(2):
        nc.sync.dma_start(
            out=fst0[:, ci, 0:CK], in_=F_s[0, 0:CK, ci * 128:(ci + 1) * 128],
            transpose=True,
        )
    for ci in range(2):
        nc.sync.dma_start(
            out=fst0[:, ci, CK:HW], in_=F_s[0, CK:HW, ci * 128:(ci + 1) * 128],
            transpose=True,
        )
    fst_t[0] = fst0

    def load_batch(b):
        """Prefetch F_a[b].T and F_s[b].T via XBAR DMA-transpose."""
        fat = qpool.tile([128, 2, T], BF16, tag="fat", name="fat")
        for ci in range(2):
            nc.sync.dma_start(
                out=fat[:, ci, :], in_=F_a[b, :, ci * 128:(ci + 1) * 128],
                transpose=True,
            )
        fat_t[b] = fat
        fst = fst_pool.tile([128, 2, HW], BF16, tag="fst", name="fst")
        for ci in range(2):
            for h in range(2):
                nc.sync.dma_start(
                    out=fst[:, ci, h * 2048:(h + 1) * 2048],
                    in_=F_s[b, h * 2048:(h + 1) * 2048, ci * 128:(ci + 1) * 128],
                    transpose=True,
                )
        fst_t[b] = fst

    def qchain(b):
        """Q~.T = Wc.T @ F_a.T + bc (scale prefolded), bf16."""
        fat = fat_t.pop(b)
        qct = qpool.tile([128, 2, T], BF16, tag="qct", name="qct")
        for m in range(2):  # d_out tile
            pj = psum_pj.tile([128, T], F32, tag="pj", name="pj")
            for k in range(2):  # d_in tile
                nc.tensor.matmul(
                    pj[:],
                    wc_sb[:, k, m * 128:(m + 1) * 128],
                    fat[:, k, :],
                    start=(k == 0),
                    stop=(k == 1),
                )
            nc.vector.tensor_scalar_add(
                out=qct[:, m, :], in0=pj[:], scalar1=bc_sb[:, m:m + 1]
            )
        qct_t[b] = qct

    def qk_chunk(b, tt, ck, s_tile):
        """QK + mask for one [128, 1024] chunk (2 PSUM banks), then exp→bf16."""
        fst = fst_t[b]
        qct = qct_t[b]
        pq = psum_qk.tile([128, CK], F32, tag="pq", name="pq")
        s0 = ck * CK
        # weight-reuse ordering: both 512-banks grouped by lhsT (qct ci)
        for ci in range(2):
            for h in range(2):  # 512-wide half = one PSUM bank
                nc.tensor.matmul(
                    pq[:, h * 512:(h + 1) * 512],
                    qct[:, ci, tt * 128:(tt + 1) * 128],
                    fst[:, ci, s0 + h * 512:s0 + (h + 1) * 512],
                    start=(ci == 0),
                    stop=False,
                )
        for h in range(2):
            mb0 = b * HW + s0 + h * 512
            nc.tensor.matmul(
                pq[:, h * 512:(h + 1) * 512],
                ones16[:],
                mb_sb[:, mb0:mb0 + 512],
                start=False,
                stop=True,
            )
        nc.scalar.activation(
            out=s_tile[:, s0:s0 + CK],
            in_=pq[:],
            func=mybir.ActivationFunctionType.Exp,
        )

    def finish_rowtile(b, tt, s_tile):
        rowsum = stats.tile([128, 1], F32, tag="rowsum", name="rowsum")
        nc.vector.reduce_sum(out=rowsum[:], in_=s_tile[:], axis=mybir.AxisListType.X)
        recip = stats.tile([128, 1], F32, tag="recip", name="recip")
        nc.vector.reciprocal(out=recip[:], in_=rowsum[:])
        o_tile = opool.tile([128, HW], BF16, tag="o", name="o")
        for h in range(2):
            sl = slice(h * 2048, (h + 1) * 2048)
            nc.vector.tensor_scalar_mul(
                out=o_tile[:, sl], in0=s_tile[:, sl], scalar1=recip[:, 0:1]
            )
            nc.sync.dma_start(
                out=S[b, tt * 128:(tt + 1) * 128, sl], in_=o_tile[:, sl]
            )

    # ---- software pipeline ----
    qchain(0)
    load_batch(1)

    for b in range(BS):
        for tt in range(2):
            s_tile = spool.tile([128, HW], BF16, tag="s", name="s")
            for ck in range(NCK):
                qk_chunk(b, tt, ck, s_tile)
                # stage prefetch + Q-chain of later batches into idle slots
                if tt == 0 and ck == 0 and b + 2 < BS:
                    load_batch(b + 2)
                elif tt == 0 and ck == 2 and b + 1 < BS:
                    qchain(b + 1)
            finish_rowtile(b, tt, s_tile)
        fst_t.pop(b, None)
        qct_t.pop(b, None)


def build_nc():
    nc = bacc.Bacc(
        "TRN2",
        target_bir_lowering=False,
        debug=False,
        num_devices=N_CORES,
    )
    F_a = nc.dram_tensor("F_a", [BS, T, D], BF16, kind="ExternalInput")
    F_s = nc.dram_tensor("F_s", [BS, HW, D], BF16, kind="ExternalInput")
    mbig = nc.dram_tensor("mbig", [BS, HW], BF16, kind="ExternalInput")
    Wc = nc.dram_tensor("Wc", [D, D], BF16, kind="ExternalInput")
    bc = nc.dram_tensor("bc", [D], F32, kind="ExternalInput")
    S = nc.dram_tensor("S", [BS, T, HW], BF16, kind="ExternalOutput")

    with tile.TileContext(nc) as tc, ExitStack() as ctx:
        _build_body(
            tc, ctx, F_a.ap(), F_s.ap(), mbig.ap(), Wc.ap(), bc.ap(), S.ap()
        )
    nc.compile()
    return nc


def make_in_maps(F_a, F_s, M_s, Wq, bq, Wk):
    F_a = np.asarray(F_a, dtype=np.float32).astype(ml_dtypes.bfloat16)
    F_s = np.asarray(F_s, dtype=np.float32).astype(ml_dtypes.bfloat16)
    M_s = np.asarray(M_s)
    Wqf = np.asarray(Wq, dtype=np.float32)
    Wkf = np.asarray(Wk, dtype=np.float32)
    bqf = np.asarray(bq, dtype=np.float32)
    # Fold: Q~ = F_a @ Wc + bc with scale pre-applied (host-side weights math)
    Wc = np.ascontiguousarray(
        ((Wqf.T @ Wkf) * np.float32(SCALE)).astype(ml_dtypes.bfloat16)
    )
    bc = np.ascontiguousarray(((bqf @ Wkf) * np.float32(SCALE)).astype(np.float32))

    m = M_s.reshape(M_s.shape[0], -1) == 1  # [B, HW]
    mbig = np.where(m, np.float32(0.0), np.float32(MASK_NEG)).astype(
        ml_dtypes.bfloat16
    )

    in_maps = []
    for i in range(N_CORES):
        sl = slice(i * BS, (i + 1) * BS)
        in_maps.append(
            dict(
                F_a=np.ascontiguousarray(F_a[sl]),
                F_s=np.ascontiguousarray(F_s[sl]),
                mbig=np.ascontiguousarray(mbig[sl]),
                Wc=Wc,
                bc=bc,
            )
        )
    return in_maps


_NC_CACHE = None


def _get_nc():
    global _NC_CACHE
    if _NC_CACHE is None:
        _NC_CACHE = build_nc()
    return _NC_CACHE


def run(in_maps, **kwargs):
    from concourse import bass_utils

    nc = _get_nc()
    res = bass_utils.run_bass_kernel_spmd(
        nc, in_maps, core_ids=list(range(N_CORES)), **kwargs
    )
    return res


def kernel(F_a, F_s, M_s, Wq, bq, Wk, bk):
    in_maps = make_in_maps(F_a, F_s, M_s, Wq, bq, Wk)
    res = run(in_maps)
    return np.concatenate(
        [np.asarray(r["S"]).astype(np.float32) for r in res.results], axis=0
    )
